# revision 14
# baseline (speedup 1.0000x reference)
"""Trainium2 Bass kernel for nn_Decoder (2-layer LSTM + Bahdanau attention + vocab
projection), data-parallel over batch on 8 NeuronCores.

Contract: kernel(**inputs) takes FULL unsharded inputs (as produced by
setup_inputs) and returns the FULL [B, T, V] float32 logits.

Design (per core, batch slice of 8 rows):
  - Recurrent weights stream from HBM in bf16 each step (they don't fit SBUF);
    Wq/Wa resident in SBUF bf16.  All matmuls bf16 x bf16 -> f32 PSUM.
  - State tensors kept transposed ("T-layout", feature-on-partition) for matmul
    lhsT; layout changes bounce through DRAM scratch with AP rearranges on the
    DRAM side (PSUM is evicted via compute-engine copies; DMA cannot read PSUM).
  - emb gather on host; x @ k0_top precomputed on device into DRAM (X0).
  - keysT = Wk.T @ memory.T precomputed on device, resident bf16.
  - Bahdanau scores: DVE add (keysT + qT broadcast) -> ACT tanh -> PE matvec
    with v; softmax on [8, 64]; ctx via per-batch PE matvecs; attention output
    projected with resident Wa; vocab projection streams Wfc bf16.
"""

import os

import numpy as np
import ml_dtypes

import concourse.bass as bass
import concourse.bacc as bacc
import concourse.mybir as mybir
from concourse import tile
from concourse.bass_utils import run_bass_kernel_spmd

FP32 = mybir.dt.float32
BF16 = mybir.dt.bfloat16
AF = mybir.ActivationFunctionType

N_CORES = 8
B, T, S = 64, int(os.environ.get("DEC_T", "32")), 64
V, E, H = 32000, 512, 1024
G = 4 * H
BL = B // N_CORES  # 8 batch rows per core
UT = H // 128      # 8 u-tiles
TP1 = T + 1


def _bf16(x):
    return np.ascontiguousarray(np.asarray(x, dtype=np.float32)).astype(
        ml_dtypes.bfloat16
    )


def build(nc, add_b0, add_b1):
    """Trace the per-core program. add_b0/add_b1: include gate-bias adds."""
    di = lambda n, s, d: nc.dram_tensor(n, s, d, kind="ExternalInput").ap()
    xT = di("xT", [E, BL * T], BF16)
    w0 = di("w0", [2 * H, G], BF16)          # [k0_bot; r0]
    w1 = di("w1", [2 * H, G], BF16)          # [k1; r1]
    k0top = di("k0top", [E, G], BF16)
    wk = di("wk", [H, H], BF16)
    wq = di("wq", [H, H], BF16)
    wa = di("wa", [2 * H, H], BF16)
    wfc = di("wfc", [H, V], BF16)
    memT = di("memT", [H, BL * S], BF16)
    memL = di("memL", [S, BL * H], BF16)  # [s, b*H + m]
    vvec = di("vvec", [128, UT], BF16)
    h0r = di("h0r", [BL, H], BF16)
    cg0 = di("cg0", [4 * BL, 256], FP32)
    id128 = di("id128", [128, 128], BF16)
    b0g = di("b0g", [4 * BL, 1024], FP32) if add_b0 else None
    b1g = di("b1g", [4 * BL, 1024], FP32) if add_b1 else None
    out = nc.dram_tensor("out", [BL * T, V], FP32, kind="ExternalOutput").ap()

    with tile.TileContext(nc) as tc:
        with (
            tc.tile_pool(name="const", bufs=1) as cpool,
            tc.tile_pool(name="state", bufs=1) as spool,
            tc.tile_pool(name="work", bufs=2) as work,
            tc.tile_pool(name="wstream", bufs=3) as wpool,
            tc.tile_pool(name="psz", bufs=4, space="PSUM") as psz,
            tc.tile_pool(name="pss", bufs=3, space="PSUM") as pss,
            tc.tile_pool(name="dram", bufs=2, space="DRAM") as dpool,
        ):
            # ---------- resident loads ----------
            keysT = cpool.tile([128, UT * BL * S], BF16, tag="keysT")  # (ut,b,s)
            memss = cpool.tile([S, BL * H], BF16, tag="memL")
            wqs = cpool.tile([128, 8 * H], BF16, tag="wqs")     # (kt, 1024)
            was = cpool.tile([128, 16 * H], BF16, tag="was")    # (kt, 1024)
            vv = cpool.tile([128, UT], BF16, tag="vv")
            ids = cpool.tile([128, 128], BF16, tag="id128")
            nc.sync.dma_start(memss[:], memL)
            nc.sync.dma_start(wqs[:], wq.rearrange("(k p) c -> p k c", p=128))
            nc.sync.dma_start(was[:], wa.rearrange("(k p) c -> p k c", p=128))
            nc.sync.dma_start(vv[:], vvec)
            nc.sync.dma_start(ids[:], id128)
            b0s = b1s = None
            if add_b0:
                b0s = cpool.tile([4 * BL, 1024], FP32, tag="b0s")
                nc.sync.dma_start(b0s[:], b0g)
            if add_b1:
                b1s = cpool.tile([4 * BL, 1024], FP32, tag="b1s")
                nc.sync.dma_start(b1s[:], b1g)

            # persistent state
            h0T = spool.tile([128, UT * BL], BF16, tag="h0T")
            h1T = spool.tile([128, UT * BL], BF16, tag="h1T")
            c0g = spool.tile([4 * BL, 256], FP32, tag="c0g")
            c1g = spool.tile([4 * BL, 256], FP32, tag="c1g")
            # attnT for steps -1..T-1 (slot t+1 = attn after step t); slot 0 = 0
            atA = spool.tile([128, UT * TP1 * BL], BF16, tag="atA")
            h0i = spool.tile([BL, H], BF16, tag="h0i")
            nc.sync.dma_start(h0i[:], h0r)
            nc.sync.dma_start(c0g[:], cg0)
            nc.sync.dma_start(c1g[:], cg0)
            nc.vector.memset(atA[:], 0.0)

            atv = atA[:].rearrange("p (u t b) -> p u t b", u=UT, t=TP1)
            h0v = h0T[:].rearrange("p (u b) -> p u b", u=UT)
            h1v = h1T[:].rearrange("p (u b) -> p u b", u=UT)

            # DRAM scratch (plain layouts; permutations live in load APs)
            X0d = dpool.tile([BL * T, G], FP32, tag="X0d", bufs=1)
            X0v = X0d[:].rearrange("(b t) (s c) -> b t s c", t=T, s=4)

            # ---------- precompute keysT and X0 (scoped pool, freed after) ----
            with tc.tile_pool(name="pre", bufs=1) as prepool:
                wks = prepool.tile([128, 8 * H], BF16, tag="wks")
                memTs = prepool.tile([128, 8 * BL * S], BF16, tag="memTs")
                xTs = prepool.tile([128, 4 * BL * T], BF16, tag="xTs")
                nc.sync.dma_start(
                    wks[:], wk.rearrange("(k p) c -> p k c", p=128))
                nc.sync.dma_start(
                    memTs[:], memT.rearrange("(k p) r -> p k r", p=128))
                nc.sync.dma_start(
                    xTs[:], xT.rearrange("(k p) r -> p k r", p=128))
                wkv = wks[:].rearrange("p (k c) -> p k c", k=8)
                mtv = memTs[:].rearrange("p (k r) -> p k r", k=8)
                kv = keysT[:].rearrange("p (u r) -> p u r", u=UT)
                for ut in range(UT):
                    kp = pss.tile([128, BL * S], FP32, tag="ps_s",
                                  name=f"kp{ut}")
                    for kt in range(8):
                        nc.tensor.matmul(
                            kp[:], wkv[:, kt, ut * 128:(ut + 1) * 128],
                            mtv[:, kt, :],
                            start=(kt == 0), stop=(kt == 7),
                        )
                    nc.any.tensor_copy(kv[:, ut, :], kp[:])  # cast -> bf16

                xv = xTs[:].rearrange("p (k r) -> p k r", k=4)
                NMT = (BL * T + 127) // 128
                for mt in range(NMT):
                    msz = min(128, BL * T - mt * 128)
                    for chb in range(2):
                        zps = [
                            psz.tile([128, 512], FP32, tag="ps_z",
                                     name=f"xzp{i}")
                            for i in range(4)
                        ]
                        for kt in range(4):
                            wt = wpool.tile([128, 2048], BF16, tag="w",
                                            name="wt")
                            nc.sync.dma_start(
                                wt[:],
                                k0top[kt * 128:(kt + 1) * 128,
                                      chb * 2048:(chb + 1) * 2048],
                            )
                            wtv = wt[:].rearrange(
                                "p (g s e) -> p g s e", g=2, s=4)
                            for sb in range(4):
                                nc.tensor.matmul(
                                    zps[sb][:, 0:msz] if False else
                                    zps[sb][0:msz, :],
                                    xv[:, kt, mt * 128:mt * 128 + msz],
                                    wtv[:, :, sb, :],
                                    start=(kt == 0), stop=(kt == 3),
                                )
                        xst = prepool.tile([128, 2048], FP32, tag="xst",
                                           name="xst", bufs=2)
                        for sb in range(4):
                            nc.any.tensor_copy(
                                xst[0:msz, sb * 512:(sb + 1) * 512],
                                zps[sb][0:msz, :],
                            )
                            nc.sync.dma_start(
                                X0d[mt * 128:mt * 128 + msz,
                                    sb * 1024 + chb * 512:
                                    sb * 1024 + chb * 512 + 512],
                                xst[0:msz, sb * 512:(sb + 1) * 512],
                            )

            # ---------- helpers ----------
            def to_T(src_b, dv, nm, nck=UT):
                """src_b sbuf bf16 [BL, nck*128] -> dv view [128, nck, BL]."""
                for ck in range(nck):
                    tp = pss.tile([128, BL], BF16, tag="ps_s",
                                  name=f"tp{nm}{ck}")
                    nc.tensor.transpose(
                        tp[:], src_b[:, ck * 128:(ck + 1) * 128],
                        ids[0:BL, 0:BL],
                    )
                    nc.any.tensor_copy(dv[:, ck, :], tp[:])

            def h_to_T(hhb, dst_T, nm):
                """hhb sbuf bf16 [32=(b,sub), 256] -> dst_T [128, (ut, b)]."""
                dv = dst_T.rearrange("p (s e b) -> p s e b", s=4, e=2)
                for e1 in range(2):
                    tp = pss.tile([128, 4 * BL], BF16, tag="ps_s",
                                  name=f"htp{nm}{e1}")
                    nc.tensor.transpose(
                        tp[:], hhb[:, e1 * 128:(e1 + 1) * 128],
                        ids[0:4 * BL, 0:4 * BL],
                    )
                    nc.any.tensor_copy(
                        dv[:, :, e1, :],
                        tp[:].rearrange("p (b s) -> p s b", b=BL),
                    )

            def lstm_z(wdram, lhs_fn, zd, nm):
                """z -> zd [8, 4096] in col2 order: addr = s*1024 + g*256 + e."""
                for chb in range(2):
                    zps = [
                        psz.tile([8, 512], FP32, tag="ps_z", name=f"z{nm}{i}")
                        for i in range(4)
                    ]
                    for kt in range(16):
                        wt = wpool.tile([128, 2048], BF16, tag="w", name="wt")
                        nc.sync.dma_start(
                            wt[:],
                            wdram[kt * 128:(kt + 1) * 128,
                                  chb * 2048:(chb + 1) * 2048],
                        )
                        wtv = wt[:].rearrange("p (g s e) -> p g s e", g=2, s=4)
                        lhs = lhs_fn(kt)
                        for sb in range(4):
                            nc.tensor.matmul(
                                zps[sb][:], lhs,
                                wtv[:, :, sb, :],
                                start=(kt == 0), stop=(kt == 15),
                            )
                    zst = work.tile([8, 2048], FP32, tag="zst", name="zst", bufs=1)
                    for sb in range(4):
                        nc.any.tensor_copy(
                            zst[:, sb * 512:(sb + 1) * 512], zps[sb][:]
                        )
                        nc.sync.dma_start(
                            zd[:, sb * 1024 + chb * 512:
                               sb * 1024 + chb * 512 + 512],
                            zst[:, sb * 512:(sb + 1) * 512],
                        )

            def gates(zd, x0_add, bias_tile, cg, hhb, nm):
                """zd DRAM [8, 4096] col2 -> gate math -> hhb bf16 [32, 256]."""
                zg = work.tile([4 * BL, 1024], FP32, tag="zg", name=f"zg{nm}")
                nc.sync.dma_start(
                    zg[:], zd.rearrange("b (s c) -> b s c", s=4)
                )
                zs = zg
                if x0_add is not None:
                    zs = work.tile([4 * BL, 1024], FP32, tag="zs", name="zs",
                                   bufs=1)
                    nc.vector.tensor_add(zs[:], zg[:], x0_add)
                if bias_tile is not None:
                    zb = work.tile([4 * BL, 1024], FP32, tag="zb", name="zb")
                    nc.vector.tensor_add(zb[:], zs[:], bias_tile[:])
                    zs = zb
                si = work.tile([4 * BL, 256], FP32, tag="si", name=f"si{nm}")
                sf = work.tile([4 * BL, 256], FP32, tag="sf", name=f"sf{nm}")
                tg = work.tile([4 * BL, 256], FP32, tag="tg", name=f"tg{nm}")
                so = work.tile([4 * BL, 256], FP32, tag="so", name=f"so{nm}")
                nc.scalar.activation(si[:], zs[:, 0:256], AF.Sigmoid)
                nc.scalar.activation(sf[:], zs[:, 256:512], AF.Sigmoid)
                nc.scalar.activation(tg[:], zs[:, 512:768], AF.Tanh)
                nc.scalar.activation(so[:], zs[:, 768:1024], AF.Sigmoid)
                m1 = work.tile([4 * BL, 256], FP32, tag="m1", name=f"m1{nm}")
                m2 = work.tile([4 * BL, 256], FP32, tag="m2", name=f"m2{nm}")
                nc.vector.tensor_mul(m1[:], sf[:], cg[:])
                nc.vector.tensor_mul(m2[:], si[:], tg[:])
                nc.vector.tensor_add(cg[:], m1[:], m2[:])
                tcc = work.tile([4 * BL, 256], FP32, tag="tcc", name=f"tc{nm}")
                nc.scalar.activation(tcc[:], cg[:], AF.Tanh)
                nc.vector.tensor_mul(hhb[:], so[:], tcc[:])

            # ---------- init h0T/h1T from enc_h ----------
            to_T(h0i[:], h0v, "init0")
            to_T(h0i[:], h1v, "init1")

            # ---------- time loop ----------
            for t in range(T):
                zd0 = dpool.tile([BL, G], FP32, tag="zd0", name=f"zd0_{t}")
                zd1 = dpool.tile([BL, G], FP32, tag="zd1", name=f"zd1_{t}")
                scd = dpool.tile([BL, S], FP32, tag="scd", name=f"scd_{t}")

                # ---- layer 0 ----
                def lhs0(kt, t=t):
                    if kt < UT:
                        return atv[:, kt, t, :]
                    return h0v[:, kt - UT, :]

                lstm_z(w0, lhs0, zd0[:], f"a{t}")
                x0s = work.tile([4 * BL, 1024], FP32, tag="x0s", name=f"x0s{t}",
                                bufs=1)
                nc.sync.dma_start(x0s[:], X0v[:, t, :, :])
                hhb0 = work.tile([4 * BL, 256], BF16, tag="hhb0",
                                 name=f"hhb0_{t}")
                gates(zd0[:], x0s[:], b0s, c0g, hhb0[:], f"a{t}")
                h_to_T(hhb0[:], h0T[:], f"h0_{t}")

                # ---- layer 1 ----
                def lhs1(kt):
                    return h0v[:, kt, :] if kt < UT else h1v[:, kt - UT, :]

                lstm_z(w1, lhs1, zd1[:], f"b{t}")
                hhb1 = work.tile([4 * BL, 256], BF16, tag="hhb1",
                                 name=f"hhb1_{t}")
                gates(zd1[:], None, b1s, c1g, hhb1[:], f"b{t}")
                h_to_T(hhb1[:], h1T[:], f"h1_{t}")

                # ---- q = h1 @ Wq ----
                wqv = wqs[:].rearrange("p (k c) -> p k c", k=8)
                qb8 = work.tile([BL, H], BF16, tag="qb8", name=f"qb8_{t}",
                                bufs=1)
                for ch in range(2):
                    qp = pss.tile([BL, 512], FP32, tag="ps_s", name=f"qp{t}{ch}")
                    for kt in range(8):
                        nc.tensor.matmul(
                            qp[:], h1v[:, kt, :],
                            wqv[:, kt, ch * 512:(ch + 1) * 512],
                            start=(kt == 0), stop=(kt == 7),
                        )
                    nc.any.tensor_copy(qb8[:, ch * 512:(ch + 1) * 512], qp[:])
                qT = work.tile([128, UT * BL], BF16, tag="qT", name=f"qT{t}")
                qv = qT[:].rearrange("p (u b) -> p u b", u=UT)
                to_T(qb8[:], qv, f"q{t}")

                # ---- scores: tanh(keysT + qT) . v ----
                addb = work.tile([128, UT * BL * S], BF16, tag="addb",
                                 name=f"addb{t}", bufs=1)
                nc.vector.tensor_add(
                    addb[:].rearrange("p (u b s) -> p u b s", u=UT, b=BL),
                    keysT[:].rearrange("p (u b s) -> p u b s", u=UT, b=BL),
                    qv.broadcast_to([128, UT, BL, S]),
                )
                thb = work.tile([128, UT * BL * S], BF16, tag="thb",
                                name=f"thb{t}", bufs=1)
                nc.scalar.activation(thb[:], addb[:], AF.Tanh)
                scp = pss.tile([1, BL * S], FP32, tag="ps_s", name=f"scp{t}")
                tv = thb[:].rearrange("p (u r) -> p u r", u=UT)
                for ut in range(UT):
                    nc.tensor.matmul(
                        scp[:], vv[:, ut:ut + 1], tv[:, ut, :],
                        start=(ut == 0), stop=(ut == 7),
                    )
                scs = work.tile([1, BL * S], FP32, tag="scs", name=f"scs{t}")
                nc.any.tensor_copy(scs[:], scp[:])
                nc.sync.dma_start(scd[:], scs[:])
                sc8 = work.tile([BL, S], FP32, tag="sc8", name=f"sc8{t}")
                nc.sync.dma_start(sc8[:], scd[:])
                mxn = work.tile([BL, 1], FP32, tag="mxn", name=f"mxn{t}")
                nc.vector.reduce_max(
                    mxn[:], sc8[:], axis=mybir.AxisListType.X, negate=True
                )
                ex = work.tile([BL, S], FP32, tag="ex", name=f"ex{t}")
                sm = work.tile([BL, 1], FP32, tag="sm", name=f"sm{t}")
                nc.scalar.activation(
                    ex[:], sc8[:], AF.Exp, bias=mxn[:], accum_out=sm[:]
                )
                rs = work.tile([BL, 1], FP32, tag="rs", name=f"rs{t}")
                nc.vector.reciprocal(rs[:], sm[:])
                al8 = work.tile([BL, S], BF16, tag="al8", name=f"al8{t}")
                nc.vector.tensor_scalar_mul(al8[:], ex[:], rs[:])
                # alS [64=s, 8=b]: transpose of al8
                tpa = pss.tile([S, BL], BF16, tag="ps_s", name=f"tpa{t}")
                nc.tensor.transpose(tpa[:], al8[:], ids[0:BL, 0:BL])
                alS = work.tile([S, BL], BF16, tag="alS", name=f"alS{t}")
                nc.any.tensor_copy(alS[:], tpa[:])

                # ---- ctx: ctxT columns via mem-as-lhsT matvecs ----
                ctxp = pss.tile([128, UT * BL], FP32, tag="ps_s",
                                name=f"ctxp{t}")
                for b in range(BL):
                    for ut in range(UT):
                        nc.tensor.matmul(
                            ctxp[:, ut * BL + b:ut * BL + b + 1],
                            memss[:, b * H + ut * 128:b * H + (ut + 1) * 128],
                            alS[:, b:b + 1],
                            start=True, stop=True,
                        )
                ctxT = work.tile([128, UT * BL], BF16, tag="ctxT",
                                 name=f"ctxT{t}")
                nc.any.tensor_copy(ctxT[:], ctxp[:])
                ctv = ctxT[:].rearrange("p (u b) -> p u b", u=UT)

                # ---- attn = [h1 | ctx] @ Wa -> atA[:, :, t+1, :] ----
                wav = was[:].rearrange("p (k c) -> p k c", k=16)
                ab8 = work.tile([BL, H], BF16, tag="ab8", name=f"ab8_{t}",
                                bufs=1)
                for ch in range(2):
                    ap_ = pss.tile([BL, 512], FP32, tag="ps_s",
                                   name=f"ap{t}{ch}")
                    for kt in range(16):
                        lhs = h1v[:, kt, :] if kt < 8 else ctv[:, kt - 8, :]
                        nc.tensor.matmul(
                            ap_[:], lhs, wav[:, kt, ch * 512:(ch + 1) * 512],
                            start=(kt == 0), stop=(kt == 15),
                        )
                    nc.any.tensor_copy(ab8[:, ch * 512:(ch + 1) * 512], ap_[:])
                to_T(ab8[:], atv[:, :, t + 1, :], f"at{t}")

            # ---------- projection: out = attn_all @ Wfc ----------
            NSC = 32  # vocab superchunks of 1000
            NPM = (T + 15) // 16  # projection M-tiles of up to 16 timesteps
            outv = out.rearrange("(b t) v -> b t v", b=BL)
            for sc_ in range(NSC):
                base = sc_ * 1000
                pps = [
                    psz.tile([128, 500], FP32, tag="ps_z", name=f"pp{sc_}{i}")
                    for i in range(2 * NPM)
                ]
                for kt in range(8):
                    wt = wpool.tile([128, 2048], BF16, tag="w", name="wt")
                    nc.sync.dma_start(
                        wt[:, 0:1000],
                        wfc[kt * 128:(kt + 1) * 128, base:base + 1000],
                    )
                    for mt in range(NPM):
                        tn = min(16, T - mt * 16)
                        lhs = atv[:, kt, mt * 16 + 1:mt * 16 + tn + 1, :]
                        for c2 in range(2):
                            nc.tensor.matmul(
                                pps[mt * 2 + c2][0:tn * BL, :], lhs,
                                wt[:, c2 * 500:(c2 + 1) * 500],
                                start=(kt == 0), stop=(kt == 7),
                            )
                for mt in range(NPM):
                    tn = min(16, T - mt * 16)
                    for c2 in range(2):
                        ost = work.tile([128, 500], FP32, tag="ost",
                                        name=f"ost{sc_}{mt}{c2}", bufs=2)
                        nc.any.tensor_copy(
                            ost[0:tn * BL, :], pps[mt * 2 + c2][0:tn * BL, :])
                        # psum [(tt,b) part, 500] -> out rows b*T + mt*16 + tt
                        nc.sync.dma_start(
                            outv[
                                :, mt * 16:mt * 16 + tn,
                                base + c2 * 500:base + (c2 + 1) * 500,
                            ].rearrange("b t v -> t b v"),
                            ost[0:tn * BL, :],
                        )
    return nc


_CACHE = {}


def _get_nc(add_b0, add_b1):
    key = (add_b0, add_b1)
    if key not in _CACHE:
        nc = bacc.Bacc(
            "TRN2", target_bir_lowering=False, debug=False,
            enable_asserts=True, num_devices=N_CORES,
        )
        build(nc, add_b0, add_b1)
        nc.compile()
        _CACHE[key] = nc
    return _CACHE[key]


def kernel(tokens, memory, enc_h, enc_c, emb, k0, r0, b0, k1, r1, b1,
           Wq, Wk, v, Wa, Wfc, bfc):
    tokens = np.asarray(tokens)
    memory = np.asarray(memory, dtype=np.float32)
    enc_h = np.asarray(enc_h, dtype=np.float32)
    enc_c = np.asarray(enc_c, dtype=np.float32)
    emb = np.asarray(emb, dtype=np.float32)
    k0 = np.asarray(k0, dtype=np.float32)
    r0 = np.asarray(r0, dtype=np.float32)
    b0 = np.asarray(b0, dtype=np.float32)
    k1 = np.asarray(k1, dtype=np.float32)
    r1 = np.asarray(r1, dtype=np.float32)
    b1 = np.asarray(b1, dtype=np.float32)
    Wq = np.asarray(Wq, dtype=np.float32)
    Wk = np.asarray(Wk, dtype=np.float32)
    v = np.asarray(v, dtype=np.float32)
    Wa = np.asarray(Wa, dtype=np.float32)
    Wfc = np.asarray(Wfc, dtype=np.float32)
    bfc = np.asarray(bfc, dtype=np.float32)

    add_b0 = bool(np.any(b0))
    add_b1 = bool(np.any(b1))
    nc = _get_nc(add_b0, add_b1)

    x = emb[tokens]  # [B, T, E] host gather
    w0h = _bf16(np.concatenate([k0[E:], r0], axis=0))
    w1h = _bf16(np.concatenate([k1, r1], axis=0))
    k0toph = _bf16(k0[:E])
    wkh, wqh, wah, wfch = _bf16(Wk), _bf16(Wq), _bf16(Wa), _bf16(Wfc)
    vh = _bf16(v.reshape(UT, 128).T)
    idh = _bf16(np.eye(128, dtype=np.float32))

    in_maps = []
    for c in range(N_CORES):
        sl = slice(c * BL, (c + 1) * BL)
        xc = x[sl]                      # [8, T, E]
        mc = memory[sl]                 # [8, S, H]
        m = {
            "xT": _bf16(xc.reshape(BL * T, E).T),
            "w0": w0h, "w1": w1h, "k0top": k0toph,
            "wk": wkh, "wq": wqh, "wa": wah, "wfc": wfch,
            "memT": _bf16(mc.reshape(BL * S, H).T),
            "memL": _bf16(mc.transpose(1, 0, 2).reshape(S, BL * H)),
            "vvec": vh,
            "h0r": _bf16(enc_h[sl]),
            "cg0": np.ascontiguousarray(
                enc_c[sl].reshape(BL * 4, 256), dtype=np.float32
            ),
            "id128": idh,
        }
        if add_b0:
            m["b0g"] = np.ascontiguousarray(np.tile(
                b0.reshape(4, 4, 256).transpose(1, 0, 2).reshape(4, 1024),
                (BL, 1)), dtype=np.float32)
        if add_b1:
            m["b1g"] = np.ascontiguousarray(np.tile(
                b1.reshape(4, 4, 256).transpose(1, 0, 2).reshape(4, 1024),
                (BL, 1)), dtype=np.float32)
        in_maps.append(m)

    res = run_bass_kernel_spmd(
        nc, in_maps, core_ids=list(range(N_CORES)), trace=False
    )
    outs = np.stack(
        [res.results[c]["out"].reshape(BL, T, V) for c in range(N_CORES)], axis=0
    ).reshape(B, T, V)
    if np.any(bfc):
        outs = outs + bfc
    return outs
